# revision 25
# baseline (speedup 1.0000x reference)
"""Bass/Trainium2 kernel for nn_Attn (dot+affect attention over encoder outputs).

Computation (per batch b):
  e[b, l] = h[b] . enc[l, b]  +  (h[b] @ affect) . emb[l, b]
  out[b, 0, :] = softmax(e[b, :])

Strategy: data-parallel over batch (8 batches per core on 8 cores). The host
packs enc and emb into one [L, BLOC*(H+A)] tensor per core so wide elementwise
multiplies + free-dim reductions compute the full dot product in one pass over
the streamed data (memory-bound regime). Engine balance per 4.2MB slab:
VectorE multiplies 6 batches in place, GpSimd 2; reductions go 2 to VectorE
(tensor_reduce) and 6 to ScalarE (activation Copy with accumulate). Softmax
runs on a transposed [128, 128] score tile using mask matmuls for
partition-group reductions/broadcasts.
"""

import numpy as np

import concourse.bass as bass
import concourse.tile as tile
from concourse import bacc, mybir
from concourse.bass_utils import run_bass_kernel_spmd

F32 = mybir.dt.float32
L, B, H, A = 2048, 64, 1024, 3
NCORES = 8
BLOC = B // NCORES          # batches per core
HE = H + A                  # extended hidden width (dot + affect features)
P = 128                     # SBUF partitions / l-tile height

N_GPS = 3                   # batches whose multiply runs on GpSimd (one wide op)
N_DVE_RED = 2               # batches whose reduction runs on VectorE


def build_nc(l_total: int = L):
    no = l_total // P       # number of l-tiles
    cols = BLOC * no        # score columns: c = b*no + o

    nc = bacc.Bacc("TRN2", target_bir_lowering=False, debug=False)

    enc_d = nc.dram_tensor("enc", [l_total, BLOC * HE], F32, kind="ExternalInput")
    hid_d = nc.dram_tensor("hid", [BLOC, H], F32, kind="ExternalInput")
    aff_d = nc.dram_tensor("aff", [H, A], F32, kind="ExternalInput")
    ident_d = nc.dram_tensor("ident", [P, P], F32, kind="ExternalInput")
    ones_d = nc.dram_tensor("ones_", [1, P], F32, kind="ExternalInput")
    bm_d = nc.dram_tensor("bm", [cols, BLOC], F32, kind="ExternalInput")
    bmT_d = nc.dram_tensor("bmT", [BLOC, cols], F32, kind="ExternalInput")
    nbmT_d = nc.dram_tensor("nbmT", [BLOC, cols], F32, kind="ExternalInput")
    sel_d = nc.dram_tensor("sel", [BLOC, BLOC * P], F32, kind="ExternalInput")
    hbx_d = nc.dram_tensor("hbx", [P, BLOC * HE], F32, kind="ExternalInput")
    out_d = nc.dram_tensor("out", [BLOC, l_total], F32, kind="ExternalOutput")

    add = mybir.AluOpType.add
    amax = mybir.AluOpType.max
    AX = mybir.AxisListType.X
    Copy = mybir.ActivationFunctionType.Copy
    Exp = mybir.ActivationFunctionType.Exp

    with tile.TileContext(nc) as tc:
        with (
            tc.tile_pool(name="const", bufs=1) as cpool,
            tc.tile_pool(name="slab", bufs=3) as spool,
            tc.tile_pool(name="scratch", bufs=2) as tpool,
            tc.tile_pool(name="ps_bc", bufs=2, space="PSUM") as ppool,
            tc.tile_pool(name="ps_sm", bufs=4, space="PSUM") as qpool,
        ):
            # ---- constants / small inputs (gpsimd DMA queue: keep the sync
            # queue free for the big streaming slabs). h and affect first —
            # they head the setup critical path. ----
            h_sb = cpool.tile([BLOC, H], F32)
            nc.gpsimd.dma_start(h_sb[:], hid_d[:])
            # affT_sb[p, ho*A+k] = affect[ho*128+p, k] — h lands on partitions
            affT_sb = cpool.tile([P, (H // P) * A], F32)
            nc.gpsimd.dma_start(
                affT_sb[:], aff_d[:].rearrange("(ho p) k -> p ho k", p=P))
            ident = cpool.tile([P, P], F32)
            nc.gpsimd.dma_start(ident[:], ident_d[:])
            ones = cpool.tile([1, P], F32)
            nc.gpsimd.dma_start(ones[:], ones_d[:])
            sel = cpool.tile([BLOC, BLOC * P], F32)
            nc.gpsimd.dma_start(sel[:], sel_d[:])
            bm = cpool.tile([cols, BLOC], F32)
            nc.gpsimd.dma_start(bm[:], bm_d[:])
            bmT = cpool.tile([BLOC, cols], F32)
            nc.gpsimd.dma_start(bmT[:], bmT_d[:])
            nbmT = cpool.tile([BLOC, cols], F32)
            nc.gpsimd.dma_start(nbmT[:], nbmT_d[:])

            # ---- ha = h @ affect ([BLOC, A]) entirely on the TensorEngine:
            # transpose h into [h-part, b] blocks via PE, then accumulate
            # K=128 matmuls against the h-partitioned affect tile. ----
            nho = H // P
            hT_sb = cpool.tile([P, nho * BLOC], F32)
            for ho in range(nho):
                hT_ps = ppool.tile([P, BLOC], F32, tag="bc", name="hT_ps")
                nc.tensor.transpose(hT_ps[:], h_sb[:, bass.ts(ho, P)],
                                    ident[0:BLOC, 0:BLOC])
                nc.scalar.copy(hT_sb[:, bass.ts(ho, BLOC)], hT_ps[:])
            ha_ps = ppool.tile([BLOC, A], F32, tag="bc", name="ha_ps")
            for ho in range(nho):
                nc.tensor.matmul(
                    ha_ps[:],
                    hT_sb[:, bass.ts(ho, BLOC)],
                    affT_sb[:, bass.ts(ho, A)],
                    start=(ho == 0), stop=(ho == nho - 1),
                )
            ha_sb = cpool.tile([BLOC, A], F32)
            nc.scalar.copy(ha_sb[:], ha_ps[:])

            # ---- hbext: the h-broadcast part arrives pre-replicated via DMA
            # (hbx, affect columns zeroed); the device fills the A-wide ha
            # columns per batch: one selector matmul per b into a [P, 8*A]
            # psum tile, then a single strided copy into place. ----
            hbext = cpool.tile([P, BLOC * HE], F32)
            nc.sync.dma_start(hbext[:], hbx_d[:])
            habx_ps = ppool.tile([P, BLOC * A], F32, tag="bc", name="habx_ps")
            for b in range(BLOC):
                nc.tensor.matmul(habx_ps[:, b * A:(b + 1) * A],
                                 sel[:, bass.ts(b, P)], ha_sb[:],
                                 start=True, stop=True)
            # hbext[:, b*HE+H : b*HE+H+A] <- habx_ps[:, b*A : (b+1)*A] for all b
            nc.scalar.copy(
                hbext[:].rearrange("p (b f) -> p b f", b=BLOC)[:, :, H:HE],
                habx_ps[:].rearrange("p (b k) -> p b k", b=BLOC),
            )

            # ---- main loop: stream enc slabs, in-place multiply, reduce ----
            scores = cpool.tile([P, cols], F32)
            enc_r = enc_d[:].rearrange("(o p) f -> o p f", p=P)   # [no, P, BLOC*HE]
            n_dve_mul = BLOC - N_GPS
            for o in range(no):
                slab = spool.tile([P, BLOC * HE], F32, tag="slab", name="slab")
                nc.sync.dma_start(slab[:], enc_r[o])
                # one wide GpSimd multiply for the tail batches (amortizes its
                # ~1.1us per-op overhead); VectorE per-batch multiplies for the
                # rest (finer-grained pipelining with the reduces)
                prodg = tpool.tile([P, N_GPS * HE], F32, tag="prodg",
                                   name="prodg", bufs=2)
                nc.gpsimd.tensor_mul(prodg[:],
                                     slab[:, n_dve_mul * HE:BLOC * HE],
                                     hbext[:, n_dve_mul * HE:BLOC * HE])
                prods = {}
                for b in range(n_dve_mul):
                    prod = tpool.tile([P, HE], F32, tag="prod", name="prod",
                                      bufs=4)
                    nc.vector.tensor_mul(prod[:], slab[:, b * HE:(b + 1) * HE],
                                         hbext[:, b * HE:(b + 1) * HE])
                    prods[b] = prod
                for b in range(BLOC):
                    c = b * no + o
                    if b < n_dve_mul:
                        pseg = prods[b][:]
                    else:
                        g = b - n_dve_mul
                        pseg = prodg[:, g * HE:(g + 1) * HE]
                    if b < N_DVE_RED:
                        nc.vector.tensor_reduce(scores[:, c:c + 1], pseg,
                                                axis=AX, op=add)
                    else:
                        cpy = tpool.tile([P, HE], F32, tag="cpy", name="cpy",
                                         bufs=2)
                        nc.scalar.activation(cpy[:], pseg, Copy,
                                             accum_out=scores[:, c:c + 1])

            # ---- softmax over l per batch, on transposed scores ----
            scT_ps = qpool.tile([cols, P], F32, tag="sm", name="scT_ps")
            nc.tensor.transpose(scT_ps[:], scores[:], ident[:])
            scT = cpool.tile([cols, P], F32)
            nc.scalar.copy(scT[:], scT_ps[:])

            rowmax = cpool.tile([cols, 1], F32)
            nc.vector.tensor_reduce(rowmax[:], scT[:], axis=AX, op=amax)
            rmT_ps = qpool.tile([1, cols], F32, tag="sm", name="rmT_ps")
            nc.tensor.matmul(rmT_ps[:], rowmax[:], ident[0:cols, 0:cols],
                             start=True, stop=True)
            rm_sb = cpool.tile([1, cols], F32)
            nc.scalar.copy(rm_sb[:], rmT_ps[:])
            bmax = cpool.tile([1, BLOC], F32)
            nc.vector.tensor_reduce(
                bmax[:], rm_sb[:].rearrange("p (b o) -> p b o", b=BLOC),
                axis=AX, op=amax)
            bcol_ps = qpool.tile([BLOC, 1], F32, tag="sm", name="bcol_ps")
            nc.tensor.matmul(bcol_ps[:], bmax[:], ones[0:1, 0:1],
                             start=True, stop=True)
            bcol = cpool.tile([BLOC, 1], F32)
            nc.scalar.copy(bcol[:], bcol_ps[:])
            negm_ps = qpool.tile([cols, 1], F32, tag="sm", name="negm_ps")
            nc.tensor.matmul(negm_ps[:], nbmT[:], bcol[:], start=True, stop=True)
            negm = cpool.tile([cols, 1], F32)
            nc.scalar.copy(negm[:], negm_ps[:])

            expT = cpool.tile([cols, P], F32)
            rowsum = cpool.tile([cols, 1], F32)
            nc.scalar.activation(expT[:], scT[:], Exp, bias=negm[:], scale=1.0,
                                 accum_out=rowsum[:])
            ssum_ps = qpool.tile([BLOC, 1], F32, tag="sm", name="ssum_ps")
            nc.tensor.matmul(ssum_ps[:], bm[:], rowsum[:], start=True, stop=True)
            rsum = cpool.tile([BLOC, 1], F32)
            nc.vector.reciprocal(rsum[:], ssum_ps[:])
            rbc_ps = qpool.tile([cols, 1], F32, tag="sm", name="rbc_ps")
            nc.tensor.matmul(rbc_ps[:], bmT[:], rsum[:], start=True, stop=True)
            rbc = cpool.tile([cols, 1], F32)
            nc.scalar.copy(rbc[:], rbc_ps[:])

            outT = cpool.tile([cols, P], F32)
            nc.vector.tensor_scalar_mul(outT[:], expT[:], rbc[:, 0:1])
            nc.sync.dma_start(out_d[:].rearrange("b (o li) -> (b o) li", o=no),
                              outT[:])

    nc.compile()
    return nc


def make_aux(l_total: int = L):
    no = l_total // P
    cols = BLOC * no
    ident = np.eye(P, dtype=np.float32)
    ones_ = np.ones((1, P), dtype=np.float32)
    bmT = np.zeros((BLOC, cols), dtype=np.float32)
    for b in range(BLOC):
        bmT[b, b * no:(b + 1) * no] = 1.0
    sel = np.zeros((BLOC, BLOC * P), dtype=np.float32)
    for b in range(BLOC):
        sel[b, b * P:(b + 1) * P] = 1.0
    return {
        "sel": sel,
        "ident": ident,
        "ones_": ones_,
        "bm": np.ascontiguousarray(bmT.T),
        "bmT": bmT,
        "nbmT": -bmT,
    }


def make_in_maps(hidden, encoder_outputs, embedding, affect_matrix, l_total: int = L):
    aux = make_aux(l_total)
    aff = np.ascontiguousarray(affect_matrix, dtype=np.float32)
    in_maps = []
    for i in range(NCORES):
        bs = slice(i * BLOC, (i + 1) * BLOC)
        enc_ext = np.concatenate(
            [encoder_outputs[:, bs, :], embedding[:, bs, :]], axis=2
        ).reshape(l_total, BLOC * HE)
        hid_loc = np.ascontiguousarray(hidden[0, bs, :], dtype=np.float32)
        hbx = np.zeros((P, BLOC * HE), dtype=np.float32)
        for b in range(BLOC):
            hbx[:, b * HE:b * HE + H] = hid_loc[b]
        in_maps.append({
            "enc": np.ascontiguousarray(enc_ext, dtype=np.float32),
            "hid": hid_loc,
            "aff": aff,
            "hbx": hbx,
            **aux,
        })
    return in_maps


_NC_CACHE = {}


def kernel(hidden, encoder_outputs, embedding, affect_matrix):
    hidden = np.asarray(hidden, dtype=np.float32)
    encoder_outputs = np.asarray(encoder_outputs, dtype=np.float32)
    embedding = np.asarray(embedding, dtype=np.float32)
    affect_matrix = np.asarray(affect_matrix, dtype=np.float32)

    if L not in _NC_CACHE:
        _NC_CACHE[L] = build_nc(L)
    nc = _NC_CACHE[L]
    in_maps = make_in_maps(hidden, encoder_outputs, embedding, affect_matrix, L)
    res = run_bass_kernel_spmd(nc, in_maps, list(range(NCORES))).results
    out = np.concatenate(
        [res[i]["out"].reshape(BLOC, 1, L) for i in range(NCORES)], axis=0
    )
    return out
